# revision 13
# baseline (speedup 1.0000x reference)
"""Trainium2 Bass kernel for nn_MemLayer (retrieval_knn).

Math:  out[b,o] = -mean_d (x[b,d] - w[o,d])^2 + bias[o]
              =  s * (x' @ w'.T)[b,o]  -  ||x_b||^2/D  +  (bias[o] - ||w_o||^2/D)

  with x' = 16*x, w' = 4096*w in fp8e4m3 and s = 2/(D*16*4096) applied on the
  ACT engine at PSUM eviction (the pre-scales keep the fp8 operands inside the
  e4m3 normal range; accumulation is fp32 in PSUM).

Strategy:
  - 2D shard across the 8 NeuronCores: batch split 4 ways x output-features
    split 2 ways. Each core computes a [2048, 2048] output tile from a 2MB x
    shard and a 2MB w shard -- this minimizes per-core DMA bytes (the kernel
    is jointly PE/DMA-limited at ~358 GB/s per core). No cross-core
    communication; outputs are gathered on the host.
  - Per core: fp8 GEMM [2048,1024] @ [1024,2048] with DoubleRow perf mode
    (2 fp8 weights per PE cell -> contraction 256 per matmul, 256 matmuls).
  - Schedule: n-tile outer; within an n-tile the contraction loop runs OUTER
    across 4-bank quarter-passes of PSUM, so the PE starts after only a few
    hundred KB of DMA and the serial ACT eviction chain of one quarter hides
    behind the next quarter's matmuls. Weight n-chunks prefetch just-in-time
    from inside the loop so they never sit ahead of output evictions in the
    shared HWDGE FIFOs. ~60 zero-matmuls burn the DMA head to hold the PE's
    HAM clock gate open so real matmuls start at 2.4 GHz.
  - Corrections stay fp32, fused into PSUM eviction:
      * ACT:  out_sb = psum * s + xsq[p]   (per-partition bias, -||x||^2/D)
      * DVE:  out_sb += v[o]               (v = bias - ||w||^2/D, row bcast)
    then one 1MB DMA per quarter straight to DRAM (per-tile DMAs on the ACT
    engine's queue for the final quarter to shorten the kernel tail).

The rank-1 reductions (x_sq, w_sq) are computed on the host in fp32, so the
only reduced-precision term is the (2/D)*x.w correction, which is ~1e-3 of
the output scale; elementwise output error stays ~2e-4 relative.
"""

import numpy as np
import ml_dtypes

B, D, O = 8192, 1024, 4096
NCORES = 8
RSPLIT = 4           # batch split
CSPLIT = 2           # output-feature split
BL = B // RSPLIT     # 2048 rows per core
OL = O // CSPLIT     # 2048 out-features per core
P = 128
MT = BL // P         # 16 m-tiles
NTILE = 512          # one PSUM bank of fp32
NT = OL // NTILE     # 4 n-tiles
QM = 4               # m-tiles per PSUM quarter-pass
KD = D // (2 * P)    # 4 double-k-tiles (fp8 DoubleRow)
XSCALE = 16.0        # x -> fp8 pre-scale
WSCALE = 4096.0      # w -> fp8 pre-scale

_CACHE = {}


def _get_nc():
    if "nc" in _CACHE:
        return _CACHE["nc"]

    import concourse.bacc as bacc
    import concourse.tile as tile
    from concourse import mybir

    nc = bacc.Bacc("TRN2", target_bir_lowering=False)

    f32 = mybir.dt.float32
    mm_dt = mybir.dt.float8e4

    xk_d = nc.dram_tensor("xk", [P, KD, 2, BL], mm_dt, kind="ExternalInput")
    wk_d = nc.dram_tensor("wk", [NT, P, KD, 2, NTILE], mm_dt,
                          kind="ExternalInput")
    xsq_d = nc.dram_tensor("xsq", [P, MT], f32, kind="ExternalInput")
    v_d = nc.dram_tensor("v", [1, OL], f32, kind="ExternalInput")
    out_d = nc.dram_tensor("out", [P, MT, OL], f32, kind="ExternalOutput")

    act_scale = float(2.0 / (D * XSCALE * WSCALE))

    with tile.TileContext(nc) as tc:
        with (
            tc.tile_pool(name="const", bufs=1) as cpool,
            tc.tile_pool(name="psum", bufs=8, space="PSUM") as ppool,
            tc.tile_pool(name="outp", bufs=4) as opool,
        ):
            xk_sb = cpool.tile([P, KD, 2, BL], mm_dt)
            wk_sb = cpool.tile([P, NT, KD, 2, NTILE], mm_dt)
            xsq_sb = cpool.tile([P, MT], f32)
            vb_sb = cpool.tile([P, OL], f32)

            # HAM warm-up while the first input chunks DMA in.
            zk = cpool.tile([P, 2, 64], mm_dt)
            nc.gpsimd.memset(zk[:], 0.0)
            ps_warm = ppool.tile([P, NTILE], f32, tag="ps")
            for _ in range(60):
                nc.tensor.matmul(
                    ps_warm[:64, :64],
                    lhsT=zk[:],
                    rhs=zk[:],
                    start=True,
                    stop=True,
                    perf_mode=mybir.MatmulPerfMode.DoubleRow,
                )

            # xk chunks enqueue on the Activation engine's DGE rings so they
            # don't serialize behind the Sync-issued weight chunks (~600ns
            # enqueue each); both streams start in parallel.
            for kc in range(KD):
                nc.scalar.dma_start(out=xk_sb[:, kc, :, :], in_=xk_d[:, kc])
                nc.sync.dma_start(out=wk_sb[:, 0, kc, :, :], in_=wk_d[0, :, kc])
            nc.sync.dma_start(out=xsq_sb[:], in_=xsq_d[:])
            nc.sync.dma_start(out=wk_sb[:, 1], in_=wk_d[1])
            nc.sync.dma_start(out=vb_sb[:], in_=v_d[:].to_broadcast([P, OL]))

            # Quarter-passes: the PE accumulates into 4 PSUM banks while the
            # ACT/DVE chain drains the previous 4 (serial ACT frees banks at
            # ~0.7us/bank, slower than the PE's first-kc-pass consumption).
            for nt in range(NT):
                if nt + 2 < NT:
                    nc.sync.dma_start(out=wk_sb[:, nt + 2], in_=wk_d[nt + 2])
                ns = slice(nt * NTILE, (nt + 1) * NTILE)
                for q in range(MT // QM):
                    mts = range(q * QM, (q + 1) * QM)
                    pss = {}
                    for mt in mts:
                        ps = ppool.tile([P, NTILE], f32, tag="ps")
                        pss[mt] = ps
                    for kc in range(KD):
                        for mt in mts:
                            nc.tensor.matmul(
                                pss[mt][:],
                                lhsT=xk_sb[:, kc, :, mt * P:(mt + 1) * P],
                                rhs=wk_sb[:, nt, kc, :, :],
                                start=(kc == 0),
                                stop=(kc == KD - 1),
                                perf_mode=mybir.MatmulPerfMode.DoubleRow,
                            )
                    if nt == NT - 1 and q == MT // QM - 1:
                        # Final quarter: per-tile eviction DMAs on the ACT
                        # engine's queue so the kernel tail is a short chain
                        # rather than one 1MB transfer gated on all 4 adds.
                        for mt in mts:
                            obs = opool.tile([P, NTILE], f32, tag="obs")
                            nc.scalar.activation(
                                obs[:],
                                pss[mt][:],
                                mybir.ActivationFunctionType.Identity,
                                bias=xsq_sb[:, mt:mt + 1],
                                scale=act_scale,
                            )
                            nc.vector.tensor_add(obs[:], obs[:], vb_sb[:, ns])
                            nc.scalar.dma_start(out=out_d[:, mt, ns], in_=obs[:])
                    else:
                        ob = opool.tile([P, QM, NTILE], f32)
                        for j, mt in enumerate(mts):
                            nc.scalar.activation(
                                ob[:, j, :],
                                pss[mt][:],
                                mybir.ActivationFunctionType.Identity,
                                bias=xsq_sb[:, mt:mt + 1],
                                scale=act_scale,
                            )
                            nc.vector.tensor_add(ob[:, j, :], ob[:, j, :],
                                                 vb_sb[:, ns])
                        mt0 = q * QM
                        nc.sync.dma_start(out=out_d[:, mt0:mt0 + QM, ns],
                                          in_=ob[:])

    nc.finalize()
    _CACHE["nc"] = nc
    return nc


def _prep_inputs(x, weights, bias):
    """Shard + lay out host inputs -> per-core in_maps (core = r * CSPLIT + c)."""
    x = np.asarray(x, dtype=np.float32)
    weights = np.asarray(weights, dtype=np.float32)
    bias = np.asarray(bias, dtype=np.float32)
    dt = ml_dtypes.float8_e4m3

    w_sq = np.einsum("od,od->o", weights, weights)
    v = (bias - w_sq / np.float32(D)).reshape(1, O)

    # k = kd*256 + i*128 + p for the DoubleRow pairing
    wT = weights.T * np.float32(WSCALE)                       # [D, O]
    wks = []
    for c in range(CSPLIT):
        wc = wT[:, c * OL:(c + 1) * OL]
        wks.append(np.ascontiguousarray(
            wc.reshape(KD, 2, P, NT, NTILE).transpose(3, 2, 0, 1, 4).astype(dt)
        ))

    xks, xsqs = [], []
    for r in range(RSPLIT):
        xs = x[r * BL:(r + 1) * BL]                           # [BL, D] fp32
        xT = xs.T                                             # [D, BL]
        xks.append(np.ascontiguousarray(
            (xT.reshape(KD, 2, P, BL) * np.float32(XSCALE))
            .transpose(2, 0, 1, 3)
            .astype(dt)
        ))
        xsq = -np.einsum("bd,bd->b", xs, xs) / np.float32(D)  # [BL]
        xsqs.append(np.ascontiguousarray(xsq.reshape(MT, P).T))  # [P, MT]

    in_maps = []
    for r in range(RSPLIT):
        for c in range(CSPLIT):
            in_maps.append({
                "xk": xks[r],
                "wk": wks[c],
                "xsq": xsqs[r],
                "v": np.ascontiguousarray(v[:, c * OL:(c + 1) * OL]),
            })
    return in_maps


def _gather(results):
    out = np.empty((B, O), dtype=np.float32)
    for r in range(RSPLIT):
        for c in range(CSPLIT):
            o = results[r * CSPLIT + c]["out"]                # [P, MT, OL]
            out[r * BL:(r + 1) * BL, c * OL:(c + 1) * OL] = (
                o.transpose(1, 0, 2).reshape(BL, OL)
            )
    return out


def _run(in_maps, **kwargs):
    from concourse.bass_utils import run_bass_kernel_spmd

    nc = _get_nc()
    return run_bass_kernel_spmd(nc, in_maps, core_ids=list(range(NCORES)), **kwargs)


def kernel(x, weights, bias):
    in_maps = _prep_inputs(x, weights, bias)
    res = _run(in_maps)
    return _gather(res.results)
